# revision 3
# baseline (speedup 1.0000x reference)
"""GQA attention (RoPE + causal softmax + o_proj) on 8 Trainium2 NeuronCores.

Sharding: core = b*4 + g where b = batch (2), g = head-group (4).
Each core handles 8 query heads (global 8g..8g+7) and their 2 KV heads
(2g, 2g+1) for one batch element, producing a partial o_proj output
(contraction over its 512 of the 2048 hd dims). The host sums the 4
partials per batch element (o_part is bf16; host upcasts).

Per-core data layout (all matmul operands bf16, fp32 PSUM accumulation):
  - x arrives pre-transposed (hid, S); xT DMA'd in 512-col chunks so the
    first projection starts after ~2MB instead of 8MB.
  - q^T/k^T built per 128-row chunk pairing heads (i, i+4); scores are
    computed transposed (S^T[k,q]) as two row-tiled K=64 matmuls that run
    concurrently in the PE array.
  - AV stationary vnat[kb] = [v0(0:64) | 1 | 0*63 | v1(128:192)]:
      av0 = vnat[:,0:65].T  @ pt0 -> v0 at partitions 0:64, den0 at 64
      av1 = vnat[:,64:192].T @ pt1 -> den1 at partition 0, v1 at 64:128
    so attnT rows 64:128 are written lane-aligned (no SBUF->SBUF DMA) and
    den1 needs no partition-move before reciprocal/broadcast.
  - At pg end avs are evicted to SBUF immediately (frees the 2 "av" PSUM
    banks for the next pg's AV accumulation), then den->recip->broadcast->
    mul chains run off SBUF.
  - Schedule: proj(0) dense, then attention chunk c interleaves proj(c+1)
    + o_proj(c-1) units as PE filler (own "fil" PSUM slots, 2 banks) so
    ScalarE exp (the per-kb rate limiter) starts ~70us earlier and PE
    never drains. PSUM: st 2x2 + av 2 + fil 2 = 8 banks.
  - Engine balance: exp exclusively on ACT; rope qraw/t1 on DVE, rope
    t2/add on GPSIMD; diagonal causal masks on GPSIMD; evictions on DVE.
"""

import numpy as np
import ml_dtypes
from contextlib import ExitStack

import concourse.mybir as mybir
from concourse import bacc
from concourse.tile import TileContext
from concourse.bass_utils import run_bass_kernel_spmd

BF16 = mybir.dt.bfloat16
F32 = mybir.dt.float32
NP_BF16 = ml_dtypes.bfloat16

HID = 2048
D = 64
H = 32           # global query heads
KV = 8           # global kv heads
B = 2
P = 128
SC = 512         # q-chunk width (also matmul free dim / PSUM bank)

_CACHE = {}


def build_nc(S):
    assert S % SC == 0
    NHID = HID // P       # hid chunks (16)
    NSB = S // P          # 128-row s-blocks
    NSC = S // SC         # 512-col s-chunks
    QCH = 4               # q chunk-pairs
    EXP = mybir.ActivationFunctionType.Exp

    nc = bacc.Bacc("TRN2", target_bir_lowering=False, debug=False)
    xT = nc.dram_tensor("xT", [HID, S], BF16, kind="ExternalInput")
    wqkv = nc.dram_tensor("wqkv", [HID, 640], BF16, kind="ExternalInput")
    wv = nc.dram_tensor("wv", [HID, 128], BF16, kind="ExternalInput")
    wo = nc.dram_tensor("wo", [512, HID], BF16, kind="ExternalInput")
    cosT = nc.dram_tensor("cosT", [128, S], BF16, kind="ExternalInput")
    sinT = nc.dram_tensor("sinT", [128, S], BF16, kind="ExternalInput")
    trimask = nc.dram_tensor("trimask", [128, 128], BF16, kind="ExternalInput")
    o_part = nc.dram_tensor("o_part", [S, HID], BF16, kind="ExternalOutput")

    with TileContext(nc) as tc, ExitStack() as ctx:
        res = ctx.enter_context(tc.tile_pool(name="res", bufs=1))
        rope = ctx.enter_context(tc.tile_pool(name="rope", bufs=2))
        ptp = ctx.enter_context(tc.tile_pool(name="ptp", bufs=6))
        nrm = ctx.enter_context(tc.tile_pool(name="nrm", bufs=1))
        obp = ctx.enter_context(tc.tile_pool(name="obp", bufs=4))
        psum = ctx.enter_context(tc.tile_pool(name="psum", bufs=1, space="PSUM"))

        # ---- input DMA: proj(0)-critical stream first ----
        # per h: weights + the s-chunk-0 slice of xT, so the first
        # projection unit is runnable after ~5MB instead of ~11MB; the
        # remaining xT col-chunks stream in behind (consumers depend at
        # slice granularity).
        xt_sb, wqkv_sb, wv_sb = [], [], []
        cos_sb = sin_sb = mask_sb = None
        for h in range(NHID):
            t = res.tile([P, S], BF16, tag=f"xt{h}", name=f"xt{h}")
            nc.sync.dma_start(out=t[:, 0:SC], in_=xT[h * P:(h + 1) * P, 0:SC])
            xt_sb.append(t)
            t = res.tile([P, 640], BF16, tag=f"wqkv{h}", name=f"wqkv{h}")
            nc.sync.dma_start(out=t, in_=wqkv[h * P:(h + 1) * P, :])
            wqkv_sb.append(t)
            t = res.tile([P, 128], BF16, tag=f"wv{h}", name=f"wv{h}")
            nc.sync.dma_start(out=t, in_=wv[h * P:(h + 1) * P, :])
            wv_sb.append(t)
            if h == 3:
                cos_sb = res.tile([P, S], BF16, tag="cos")
                nc.sync.dma_start(out=cos_sb, in_=cosT[:, :])
                sin_sb = res.tile([P, S], BF16, tag="sin")
                nc.sync.dma_start(out=sin_sb, in_=sinT[:, :])
                mask_sb = res.tile([P, P], BF16, tag="mask")
                nc.sync.dma_start(out=mask_sb, in_=trimask[:, :])
        for s in range(1, NSC):
            sl = slice(s * SC, (s + 1) * SC)
            for h in range(NHID):
                nc.sync.dma_start(out=xt_sb[h][:, sl], in_=xT[h * P:(h + 1) * P, sl])
        wo_sb = []
        for i in range(4):
            t = res.tile([P, HID], BF16, tag=f"wo{i}", name=f"wo{i}")
            nc.sync.dma_start(out=t, in_=wo[i * P:(i + 1) * P, :])
            wo_sb.append(t)

        # chunks 0-3: q head pairs (i, i+4); chunk 4: k (kv0 rows 0-63, kv1 64-127)
        qkrot = []
        for m in range(5):
            t = res.tile([P, S], BF16, tag=f"qkrot{m}", name=f"qkrot{m}")
            qkrot.append(t)
        # v tiles [128, 192]: [v0(0:64) | 1 | 0*63 | v1(128:192)]
        vnat = [res.tile([P, 192], BF16, tag=f"vnat{sb}", name=f"vnat{sb}")
                for sb in range(NSB)]
        attnT = []
        for i in range(QCH):
            t = res.tile([P, S], BF16, tag=f"attnT{i}", name=f"attnT{i}")
            attnT.append(t)

        def emit_o_unit(qb, n):
            po = psum.tile([P, SC], F32, tag="fil", bufs=2, name="po")
            for i in range(QCH):
                nc.tensor.matmul(
                    po,
                    lhsT=attnT[i][:, qb * P:(qb + 1) * P],
                    rhs=wo_sb[i][:, n * SC:(n + 1) * SC],
                    start=(i == 0),
                    stop=(i == QCH - 1),
                )
            ob = obp.tile([P, SC], BF16, tag="ob", name="ob")
            nc.vector.tensor_copy(ob, po)
            nc.sync.dma_start(
                out=o_part[qb * P:(qb + 1) * P, n * SC:(n + 1) * SC], in_=ob
            )

        def gen_proj_schunk(s):
            """Emit s-chunk s projections + RoPE + v as units (yields).

            Matmul groups stay consecutive; eviction units only follow
            completed groups. Rope eviction split: qraw/t1 on DVE (both
            only need ps), t2 and the final add on GPSIMD so the DVE
            FIFO never head-of-line-blocks on the qswp DMA chain."""
            sl = slice(s * SC, (s + 1) * SC)
            for m in (4, 0, 1, 2, 3):
                ps = psum.tile([P, SC], F32, tag="fil", bufs=2, name="ps_proj")
                for h0 in (0, 8):
                    for h in range(h0, h0 + 8):
                        nc.tensor.matmul(
                            ps,
                            lhsT=wqkv_sb[h][:, m * P:(m + 1) * P],
                            rhs=xt_sb[h][:, sl],
                            start=(h == 0),
                            stop=(h == NHID - 1),
                        )
                    yield
                # rotate_half operand: engines are lane-locked, so the
                # +-32-partition swap must go through DMA (SBUF->SBUF)
                qraw = rope.tile([P, SC], BF16, tag="qraw", bufs=2, name="qraw")
                nc.vector.tensor_copy(qraw, ps)
                qswp = rope.tile([P, SC], BF16, tag="qswp", bufs=2, name="qswp")
                for dst, src in ((0, 32), (32, 0), (64, 96), (96, 64)):
                    nc.sync.dma_start(
                        out=qswp[dst:dst + 32, :], in_=qraw[src:src + 32, :]
                    )
                t1 = rope.tile([P, SC], BF16, tag="t1", bufs=2, name="t1")
                nc.vector.tensor_mul(t1, ps, cos_sb[:, sl])
                t2 = rope.tile([P, SC], BF16, tag="t2", bufs=2, name="t2")
                nc.gpsimd.tensor_mul(t2, qswp, sin_sb[:, sl])
                nc.gpsimd.tensor_add(qkrot[m][:, sl], t1, t2)
                yield
            for sb in range(4 * s, 4 * s + 4):
                t = vnat[sb]
                nc.gpsimd.memset(t[:, 64:65], 1.0)
                nc.gpsimd.memset(t[:, 65:128], 0.0)
                pv = psum.tile([P, 128], F32, tag="fil", bufs=2, name="ps_v")
                for h in range(NHID):
                    nc.tensor.matmul(
                        pv,
                        lhsT=xt_sb[h][:, sb * P:(sb + 1) * P],
                        rhs=wv_sb[h],
                        start=(h == 0),
                        stop=(h == NHID - 1),
                    )
                yield
                nc.vector.tensor_copy(t[:, 0:64], pv[:, 0:64])
                nc.vector.tensor_copy(t[:, 128:192], pv[:, 64:128])
                yield

        def gen_o_chunk(c):
            for qb in range(4 * c, 4 * c + 4):
                for n in range(HID // SC):
                    emit_o_unit(qb, n)
                    yield

        def interleave(*gens):
            gens = [g for g in gens if g is not None]
            i = 0
            while gens:
                g = gens[i % len(gens)]
                try:
                    next(g)
                except StopIteration:
                    gens.remove(g)
                    continue
                yield
                i += 1

        # ---- proj chunk 0 dense (nothing to overlap with yet) ----
        with nc.named_scope("projA0"):
            for _ in gen_proj_schunk(0):
                pass

        # ---- attention chunks; proj(c+1) and o(c-1) drip into the kb
        # loop as PE fill work while ScalarE streams exps ----
        for c in range(NSC):
          with nc.named_scope(f"attn_c{c}"):
            q0 = c * SC
            nkb = 4 * c + 4
            filler = interleave(
                gen_proj_schunk(c + 1) if c + 1 < NSC else None,
                gen_o_chunk(c - 1) if c >= 1 else None,
            )
            n_units = (23 if c + 1 < NSC else 0) + (16 if c >= 1 else 0)
            total_iters = 4 * nkb
            it = 0
            spent = 0
            for pg in (0, 1, 2, 3):
                av0 = psum.tile([P, SC], F32, tag="av", bufs=2, name="av0")
                av1 = psum.tile([P, SC], F32, tag="av", bufs=2, name="av1")
                hp = pg

                def emit_av(kb, pt, vs):
                    nc.tensor.matmul(
                        av0[0:65, vs:SC],
                        lhsT=vnat[kb][:, 0:65],
                        rhs=pt[:, vs:SC],
                        start=(kb == 0), stop=(kb == nkb - 1),
                    )
                    nc.tensor.matmul(
                        av1[:, vs:SC],
                        lhsT=vnat[kb][:, 64:192],
                        rhs=pt[:, SC + vs:2 * SC],
                        start=(kb == 0), stop=(kb == nkb - 1),
                    )

                # two-stage software pipeline: AV(kb-3) is emitted after
                # scores(kb), giving each exp ~two iterations of cover
                pending = []
                for kb in range(nkb):
                    vs = max(0, (kb - 4 * c) * P)  # first valid col in chunk
                    st = psum.tile([P, 2 * SC], F32, tag="st", bufs=2, name="st")
                    nc.tensor.matmul(
                        st[:, vs:SC],
                        lhsT=qkrot[4][0:64, kb * P:(kb + 1) * P],
                        rhs=qkrot[hp][0:64, q0 + vs:q0 + SC],
                        start=True, stop=True,
                    )
                    nc.tensor.matmul(
                        st[:, SC + vs:2 * SC],
                        lhsT=qkrot[4][64:128, kb * P:(kb + 1) * P],
                        rhs=qkrot[hp][64:128, q0 + vs:q0 + SC],
                        start=True, stop=True,
                    )
                    if len(pending) >= 3:
                        emit_av(*pending.pop(0))
                    pt = ptp.tile([P, 2 * SC], BF16, tag="pt", name="pt")
                    # one exp over [vs:1024]: the dead span [SC:SC+vs] is
                    # unwritten PSUM (may exp to junk; never read)
                    nc.scalar.activation(
                        pt[:, vs:2 * SC], st[:, vs:2 * SC], EXP, scale=0.125
                    )
                    if kb - 4 * c >= 0:  # diagonal block: mask triangle
                        nc.gpsimd.tensor_mul(
                            pt[:, vs:vs + P], pt[:, vs:vs + P], mask_sb
                        )
                        nc.gpsimd.tensor_mul(
                            pt[:, SC + vs:SC + vs + P],
                            pt[:, SC + vs:SC + vs + P], mask_sb
                        )
                    pending.append((kb, pt, vs))
                    it += 1
                    want = (it * n_units) // total_iters
                    while spent < want:
                        try:
                            next(filler)
                            spent += 1
                        except StopIteration:
                            spent = want
                            break
                for pp in pending:
                    emit_av(*pp)
                    try:
                        next(filler)
                        spent += 1
                    except StopIteration:
                        pass

                # ---- normalize: evict avs to SBUF first (frees the av
                # PSUM banks for the next pg), then den->recip->broadcast->
                # mul off SBUF. B-chain (den1 at partition 0, no DMA) is
                # emitted so its DVE ops run while dA's DMA is in flight.
                av0e = nrm.tile([65, SC], F32, tag="av0e", bufs=1, name="av0e")
                nc.vector.tensor_copy(av0e, av0[0:65, :])
                av1e = nrm.tile([P, SC], F32, tag="av1e", bufs=1, name="av1e")
                nc.vector.tensor_copy(av1e, av1)
                dA = nrm.tile([1, SC], F32, tag="dA", bufs=1, name="dA")
                nc.sync.dma_start(out=dA, in_=av0e[64:65, :])
                rB = nrm.tile([1, SC], F32, tag="rB", bufs=1, name="rB")
                nc.vector.reciprocal_approx_fast(rB, av1e[0:1, :])
                rbB = nrm.tile([P, SC], F32, tag="rbB", bufs=1, name="rbB")
                nc.gpsimd.partition_broadcast(rbB, rB)
                nc.vector.tensor_mul(
                    attnT[hp][64:128, q0:q0 + SC], av1e[64:128, :], rbB[64:128, :]
                )
                rA = nrm.tile([1, SC], F32, tag="rA", bufs=1, name="rA")
                nc.vector.reciprocal_approx_fast(rA, dA)
                rbA = nrm.tile([64, SC], F32, tag="rbA", bufs=1, name="rbA")
                nc.gpsimd.partition_broadcast(rbA, rA)
                nc.vector.tensor_mul(
                    attnT[hp][0:64, q0:q0 + SC], av0e[0:64, :], rbA
                )

            # drain remaining filler (next chunk depends on its qkrot/vnat)
            for _ in filler:
                pass
        # last chunk's o_proj tail
        for _ in gen_o_chunk(NSC - 1):
            pass

    nc.finalize()
    return nc


def prep_core_inputs(x, cos, sin, wq, wk, wv, wo, core, _shared={}):
    """Build the per-core input map (all host-side numpy)."""
    b, g = core // 4, core % 4
    S = x.shape[1]

    key = ("xT", b, id(x))
    if key not in _shared:
        _shared.clear() if len(_shared) > 8 else None
        _shared[key] = np.ascontiguousarray(x[b].T).astype(NP_BF16)
    xT = _shared[key]

    qcols = []
    for i in range(4):
        h0, h1 = 8 * g + i, 8 * g + i + 4
        qcols.append(wq[:, h0 * D:(h0 + 1) * D])
        qcols.append(wq[:, h1 * D:(h1 + 1) * D])
    kcols = wk[:, 2 * g * D:(2 * g + 2) * D]
    wqkv_c = np.concatenate(qcols + [kcols], axis=1).astype(NP_BF16)
    wv_c = np.ascontiguousarray(wv[:, 2 * g * D:(2 * g + 2) * D]).astype(NP_BF16)
    worows = []
    for i in range(4):
        h0, h1 = 8 * g + i, 8 * g + i + 4
        worows.append(wo[h0 * D:(h0 + 1) * D, :])
        worows.append(wo[h1 * D:(h1 + 1) * D, :])
    wo_c = np.concatenate(worows, axis=0).astype(NP_BF16)

    cosT = np.tile(cos[:S].T, (2, 1)).astype(NP_BF16)
    sinT_h = np.concatenate([-sin[:S].T[:D // 2], sin[:S].T[D // 2:]], axis=0)
    sinT = np.tile(sinT_h, (2, 1)).astype(NP_BF16)
    trimask = np.triu(np.ones((P, P), dtype=NP_BF16))

    return {
        "xT": xT, "wqkv": wqkv_c, "wv": wv_c, "wo": wo_c,
        "cosT": cosT, "sinT": sinT, "trimask": trimask,
    }


def kernel(x, cos, sin, wq, wk, wv, wo):
    x = np.asarray(x)
    S = x.shape[1]
    assert x.shape == (B, S, HID)
    if S not in _CACHE:
        _CACHE[S] = build_nc(S)
    nc = _CACHE[S]
    in_maps = [
        prep_core_inputs(x, np.asarray(cos), np.asarray(sin), np.asarray(wq),
                         np.asarray(wk), np.asarray(wv), np.asarray(wo), core)
        for core in range(8)
    ]
    res = run_bass_kernel_spmd(nc, in_maps, core_ids=list(range(8)))
    out = np.zeros((B, S, HID), np.float32)
    for core in range(8):
        out[core // 4] += res.results[core]["o_part"].astype(np.float32)
    return out
